# revision 2
# baseline (speedup 1.0000x reference)
"""NeighborAware GNN message-passing kernel for 8 Trainium2 NeuronCores.

Strategy (data-parallel): shard the 16384-sample batch across 8 cores
(2048 samples each); replicate the embedding tables + tiny weights.

Algebraic collapse of the single-head attention (softmax is shift
invariant, and only the first-token output is used):
    scores_j = x0^T A x_j + c1 . x_j        A  = Wq^T Wk / sqrt(E)
                                            c1 = Wk^T bq / sqrt(E)
    ctx_out  = (sum_j a_j x_j) @ M_vo + b'  M_vo = (Wo Wv)^T
so neighbor embeddings never need projection; each sample needs only its
12 gathered rows (2 sides x (target + 5 neighbors)), per-sample dot
products on DVE, and two small matmuls per 128-sample tile. The output
biases (Wo bv + out_b) are folded into the first MLP bias.

Per 128-sample tile-and-side: 6 indirect DMA gathers ([P,1]-offset form,
the only HW-supported one), one PE transpose of the target rows, the
z0 = x0 A + c1 matmul, 6 fused mul-reduce score ops (custom-DVE
TENSOR_TENSOR_REDUCE with the pad mask as the accumulator seed),
softmax via ACT Exp with fused bias/accum, 6 scaled copies + 6
accumulating PE transposes for the weighted sum, and one matmul for the
projected context. A second phase runs the 3-layer MLP transposed so no
further transposes are needed.
"""
import sys

if "/opt/trn_rl_repo" not in sys.path:
    sys.path.insert(0, "/opt/trn_rl_repo")

import numpy as np

import concourse.bass as bass
import concourse.bacc as bacc
import concourse.tile as tile
from concourse import mybir
from concourse.masks import make_identity
from concourse.dve_ops import TENSOR_TENSOR_REDUCE
from concourse.bass_utils import run_bass_kernel_spmd

N_CORES = 8
BATCH = 16384
BC = BATCH // N_CORES          # 2048 samples per core
P = 128
NTILES = BC // P               # 16 tiles per core
EMB = 128
K = 5
NJ = K + 1                     # target + 5 neighbors
V = 100001                     # rows per table (incl. padding row 0)
CATV = 2 * V                   # user and item tables concatenated

f32 = mybir.dt.float32
i32 = mybir.dt.int32
RSQRT_E = float(1.0 / np.sqrt(np.float32(EMB)))

_PROGRAM = None


def _build_program():
    nc = bacc.Bacc()

    cat_d = nc.dram_tensor("cat_table", [CATV, EMB], f32, kind="ExternalInput")
    idx_d = nc.dram_tensor("idx", [BC, 2 * NJ], i32, kind="ExternalInput")
    wdram = {}
    for s in ("u", "i"):
        wdram[f"{s}_in_w"] = nc.dram_tensor(f"{s}_in_w", [3 * EMB, EMB], f32, kind="ExternalInput")
        wdram[f"{s}_in_b"] = nc.dram_tensor(f"{s}_in_b", [3 * EMB], f32, kind="ExternalInput")
        wdram[f"{s}_out_w"] = nc.dram_tensor(f"{s}_out_w", [EMB, EMB], f32, kind="ExternalInput")
        wdram[f"{s}_out_b"] = nc.dram_tensor(f"{s}_out_b", [EMB], f32, kind="ExternalInput")
    W1_d = nc.dram_tensor("W1", [EMB, 2 * EMB], f32, kind="ExternalInput")
    b1_d = nc.dram_tensor("b1", [EMB], f32, kind="ExternalInput")
    W2_d = nc.dram_tensor("W2", [EMB // 2, EMB], f32, kind="ExternalInput")
    b2_d = nc.dram_tensor("b2", [EMB // 2], f32, kind="ExternalInput")
    W3_d = nc.dram_tensor("W3", [1, EMB // 2], f32, kind="ExternalInput")
    b3_d = nc.dram_tensor("b3", [1], f32, kind="ExternalInput")
    y_d = nc.dram_tensor("y", [BC], f32, kind="ExternalOutput")

    with tile.TileContext(nc) as tc:
        with tc.tile_pool(name="singles", bufs=1) as singles:
            ident = singles.tile([P, P], f32)
            make_identity(nc, ident[:])
            ones_row = singles.tile([1, P], f32)
            nc.vector.memset(ones_row[:], 1.0)

            # ---- load raw weights -------------------------------------
            A_s, c1_s, Mvo_s, bout_s = [], [], [], []
            with tc.tile_pool(name="wload", bufs=1) as wl, \
                 tc.tile_pool(name="setup_psum", bufs=1, space="PSUM") as sps:
                for si, s in enumerate(("u", "i")):
                    wq = wl.tile([P, P], f32, tag=f"wq{s}")
                    wk = wl.tile([P, P], f32, tag=f"wk{s}")
                    wv = wl.tile([P, P], f32, tag=f"wv{s}")
                    nc.sync.dma_start(out=wq[:], in_=wdram[f"{s}_in_w"][0:P, :])
                    nc.sync.dma_start(out=wk[:], in_=wdram[f"{s}_in_w"][P:2 * P, :])
                    nc.sync.dma_start(out=wv[:], in_=wdram[f"{s}_in_w"][2 * P:3 * P, :])
                    bq = wl.tile([P, 1], f32, tag=f"bq{s}")
                    bv = wl.tile([P, 1], f32, tag=f"bv{s}")
                    nc.sync.dma_start(out=bq[:], in_=wdram[f"{s}_in_b"][0:P, None])
                    nc.sync.dma_start(out=bv[:], in_=wdram[f"{s}_in_b"][2 * P:3 * P, None])
                    wo = wl.tile([P, P], f32, tag=f"wo{s}")
                    nc.sync.dma_start(out=wo[:], in_=wdram[f"{s}_out_w"][:, :])
                    outb = wl.tile([P, 1], f32, tag=f"ob{s}")
                    nc.sync.dma_start(out=outb[:], in_=wdram[f"{s}_out_b"][:, None])

                    # A = Wq^T Wk / sqrt(E)   [e, e']
                    A_p = sps.tile([P, P], f32, tag="sp0")
                    nc.tensor.matmul(A_p[:], lhsT=wq[:], rhs=wk[:], start=True, stop=True)
                    A_t = singles.tile([P, P], f32, tag=f"A{s}")
                    nc.vector.tensor_scalar_mul(A_t[:], A_p[:], RSQRT_E)
                    A_s.append(A_t)

                    # c1 = bq^T Wk / sqrt(E)  [1, e']
                    c1_p = sps.tile([1, P], f32, tag="sp1")
                    nc.tensor.matmul(c1_p[:], lhsT=bq[:], rhs=wk[:], start=True, stop=True)
                    c1_t = singles.tile([1, P], f32, tag=f"c1{s}")
                    nc.vector.tensor_scalar_mul(c1_t[:], c1_p[:], RSQRT_E)
                    c1_s.append(c1_t)

                    # WoT [g, f]
                    woT_p = sps.tile([P, P], f32, tag="sp0")
                    nc.tensor.transpose(woT_p[:], wo[:], ident[:])
                    woT = wl.tile([P, P], f32, tag=f"woT{s}")
                    nc.vector.tensor_copy(woT[:], woT_p[:])

                    # M_vo[e, f] = sum_g Wv[g,e] WoT[g,f]
                    mvo_p = sps.tile([P, P], f32, tag="sp0")
                    nc.tensor.matmul(mvo_p[:], lhsT=wv[:], rhs=woT[:], start=True, stop=True)
                    mvo = singles.tile([P, P], f32, tag=f"mvo{s}")
                    nc.vector.tensor_copy(mvo[:], mvo_p[:])
                    Mvo_s.append(mvo)

                    # b_out = Wo bv + out_b  [f, 1]
                    bo_p = sps.tile([P, 1], f32, tag="sp1")
                    nc.tensor.matmul(bo_p[:], lhsT=woT[:], rhs=bv[:], start=True, stop=True)
                    bo = wl.tile([P, 1], f32, tag=f"bo{s}")
                    nc.vector.tensor_add(out=bo[:], in0=bo_p[:], in1=outb[:])
                    bout_s.append(bo)

                # MLP weights (transposed for lhsT use)
                w1 = wl.tile([P, 2 * P], f32)
                nc.sync.dma_start(out=w1[:], in_=W1_d[:, :])
                w1uT_p = sps.tile([P, P], f32, tag="sp0")
                nc.tensor.transpose(w1uT_p[:], w1[:, 0:P], ident[:])
                w1uT = singles.tile([P, P], f32)
                nc.vector.tensor_copy(w1uT[:], w1uT_p[:])
                w1iT_p = sps.tile([P, P], f32, tag="sp0")
                nc.tensor.transpose(w1iT_p[:], w1[:, P:2 * P], ident[:])
                w1iT = singles.tile([P, P], f32)
                nc.vector.tensor_copy(w1iT[:], w1iT_p[:])

                w2 = wl.tile([P // 2, P], f32)
                nc.sync.dma_start(out=w2[:], in_=W2_d[:, :])
                w2T_p = sps.tile([P, P // 2], f32, tag="sp0")
                nc.tensor.matmul(w2T_p[:], lhsT=w2[:], rhs=ident[0:P // 2, 0:P // 2],
                                 is_transpose=True, start=True, stop=True)
                w2T = singles.tile([P, P // 2], f32)
                nc.vector.tensor_copy(w2T[:], w2T_p[:])

                w3c = singles.tile([P // 2, 1], f32)
                nc.sync.dma_start(out=w3c[:], in_=W3_d[0, :, None])
                b1c = wl.tile([P, 1], f32)
                nc.sync.dma_start(out=b1c[:], in_=b1_d[:, None])
                b2c = singles.tile([P // 2, 1], f32)
                nc.sync.dma_start(out=b2c[:], in_=b2_d[:, None])
                b3c = singles.tile([1, 1], f32)
                nc.sync.dma_start(out=b3c[:], in_=b3_d[:, None])

                # b1' = b1 + W1u b_out_u + W1i b_out_i
                b1p_p = sps.tile([P, 1], f32, tag="sp1")
                nc.tensor.matmul(b1p_p[:], lhsT=w1uT[:], rhs=bout_s[0][:], start=True, stop=False)
                nc.tensor.matmul(b1p_p[:], lhsT=w1iT[:], rhs=bout_s[1][:], start=False, stop=True)
                b1p = singles.tile([P, 1], f32)
                nc.vector.tensor_add(out=b1p[:], in0=b1p_p[:], in1=b1c[:])

            # context staging for phase B: [side*16 + t] slots of [P, P]
            ctx_all = singles.tile([P, 2 * NTILES, P], f32)
            y_row = singles.tile([1, BC], f32)

            # ---------------- phase A: gather + attention ----------------
            with tc.tile_pool(name="idxp", bufs=3) as idxp, \
                 tc.tile_pool(name="gp", bufs=3) as gp, \
                 tc.tile_pool(name="wp", bufs=2) as wp, \
                 tc.tile_pool(name="sp", bufs=3) as sp, \
                 tc.tile_pool(name="cp", bufs=3) as cp, \
                 tc.tile_pool(name="pa", bufs=2, space="PSUM") as pa:
                for t in range(NTILES):
                    idx_t = idxp.tile([P, 2 * NJ], i32)
                    nc.sync.dma_start(out=idx_t[:], in_=idx_d[t * P:(t + 1) * P, :])
                    for side in range(2):
                        base = side * NJ
                        xg = gp.tile([P, NJ, EMB], f32, tag=f"xg{side}")
                        for j in range(NJ):
                            nc.gpsimd.indirect_dma_start(
                                out=xg[:, j, :], out_offset=None, in_=cat_d[:, :],
                                in_offset=bass.IndirectOffsetOnAxis(
                                    ap=idx_t[:, base + j:base + j + 1], axis=0))

                        x0T_p = pa.tile([P, P], f32, tag="x0T")
                        nc.tensor.transpose(x0T_p[:], xg[:, 0, :], ident[:])
                        x0T = cp.tile([P, P], f32, tag="x0T_s")
                        nc.vector.tensor_copy(x0T[:], x0T_p[:])

                        z0_p = pa.tile([P, P], f32, tag="z0")
                        nc.tensor.matmul(z0_p[:], lhsT=x0T[:], rhs=A_s[side][:],
                                         start=True, stop=False)
                        nc.tensor.matmul(z0_p[:], lhsT=ones_row[:], rhs=c1_s[side][:],
                                         start=False, stop=True)

                        msk = sp.tile([P, K], f32, tag="msk")
                        nc.vector.tensor_scalar(
                            out=msk[:], in0=idx_t[:, base + 1:base + NJ],
                            scalar1=0, scalar2=-1e30,
                            op0=mybir.AluOpType.is_equal, op1=mybir.AluOpType.mult)

                        scores = sp.tile([P, NJ], f32, tag="sc")
                        scratch = cp.tile([P, P], f32, tag="ttr")
                        for j in range(NJ):
                            nc.vector._custom_dve(
                                TENSOR_TENSOR_REDUCE,
                                out=scratch[:], in0=z0_p[:], in1=xg[:, j, :],
                                s0=(0.0 if j == 0 else msk[:, j - 1:j]), s1=1.0,
                                accum_out=scores[:, j:j + 1])

                        negmx = sp.tile([P, 1], f32, tag="mx")
                        nc.vector.reduce_max(out=negmx[:], in_=scores[:],
                                             axis=mybir.AxisListType.X, negate=True)
                        aexp = sp.tile([P, NJ], f32, tag="ae")
                        sumex = sp.tile([P, 1], f32, tag="se")
                        nc.scalar.activation(out=aexp[:], in_=scores[:],
                                             func=mybir.ActivationFunctionType.Exp,
                                             bias=negmx[:], scale=1.0, accum_out=sumex[:])
                        rec = sp.tile([P, 1], f32, tag="rc")
                        nc.vector.reciprocal(rec[:], sumex[:])
                        anorm = sp.tile([P, NJ], f32, tag="an")
                        nc.vector.tensor_scalar_mul(anorm[:], aexp[:], rec[:])

                        wacc = wp.tile([P, NJ, EMB], f32, tag=f"wacc{side}")
                        for j in range(NJ):
                            nc.vector.tensor_scalar_mul(wacc[:, j, :], xg[:, j, :],
                                                        anorm[:, j:j + 1])
                        wT_p = pa.tile([P, P], f32, tag="wT")
                        for j in range(NJ):
                            nc.tensor.matmul(wT_p[:], lhsT=wacc[:, j, :], rhs=ident[:],
                                             is_transpose=True,
                                             start=(j == 0), stop=(j == NJ - 1))
                        wT = cp.tile([P, P], f32, tag="wT_s")
                        nc.vector.tensor_copy(wT[:], wT_p[:])

                        ctx_p = pa.tile([P, P], f32, tag="ctx")
                        nc.tensor.matmul(ctx_p[:], lhsT=Mvo_s[side][:], rhs=wT[:],
                                         start=True, stop=True)
                        nc.vector.tensor_copy(ctx_all[:, side * NTILES + t, :], ctx_p[:])

            # ---------------- phase B: MLP (transposed) ------------------
            with tc.tile_pool(name="cb", bufs=3) as cb, \
                 tc.tile_pool(name="pb", bufs=2, space="PSUM") as pb:
                for t in range(NTILES):
                    h1_p = pb.tile([P, P], f32, tag="h1")
                    nc.tensor.matmul(h1_p[:], lhsT=w1uT[:], rhs=ctx_all[:, t, :],
                                     start=True, stop=False)
                    nc.tensor.matmul(h1_p[:], lhsT=w1iT[:], rhs=ctx_all[:, NTILES + t, :],
                                     start=False, stop=True)
                    h1 = cb.tile([P, P], f32, tag="h1s")
                    nc.scalar.activation(out=h1[:], in_=h1_p[:],
                                         func=mybir.ActivationFunctionType.Relu,
                                         bias=b1p[:], scale=1.0)
                    h2_p = pb.tile([P // 2, P], f32, tag="h2")
                    nc.tensor.matmul(h2_p[:], lhsT=w2T[:], rhs=h1[:], start=True, stop=True)
                    h2 = cb.tile([P // 2, P], f32, tag="h2s")
                    nc.scalar.activation(out=h2[:], in_=h2_p[:],
                                         func=mybir.ActivationFunctionType.Relu,
                                         bias=b2c[:], scale=1.0)
                    y_p = pb.tile([1, P], f32, tag="y")
                    nc.tensor.matmul(y_p[:], lhsT=w3c[:], rhs=h2[:], start=True, stop=True)
                    nc.vector.tensor_scalar_add(y_row[:, t * P:(t + 1) * P], y_p[:], b3c[:])

            nc.sync.dma_start(out=y_d[None, :], in_=y_row[:])

    nc.compile()
    return nc


def _get_program():
    global _PROGRAM
    if _PROGRAM is None:
        _PROGRAM = _build_program()
    return _PROGRAM


def kernel(**inputs) -> np.ndarray:
    user = np.asarray(inputs["user"]).astype(np.int64)
    item = np.asarray(inputs["item"]).astype(np.int64)
    user_table = np.ascontiguousarray(np.asarray(inputs["user_table"], dtype=np.float32))
    item_table = np.ascontiguousarray(np.asarray(inputs["item_table"], dtype=np.float32))
    user_topk = np.asarray(inputs["user_topk"]).astype(np.int64)
    item_topk = np.asarray(inputs["item_topk"]).astype(np.int64)

    nv = user_table.shape[0]
    assert nv == V and user.shape[0] == BATCH, (user_table.shape, user.shape)

    cat = np.ascontiguousarray(np.concatenate([user_table, item_table], axis=0))

    # index preprocessing: resolve top-k neighbor ids for the batch and
    # fold the item-table offset in; id 0 stays 0 (padding row, masked out).
    u_ids = user_topk[user]                                   # [B, K]
    i_ids_raw = item_topk[item]                               # [B, K]
    i_ids = np.where(i_ids_raw == 0, 0, i_ids_raw + nv)
    idx_all = np.concatenate(
        [user[:, None], u_ids, item[:, None] + nv, i_ids], axis=1
    ).astype(np.int32)                                        # [B, 12]

    weights = {
        k: np.ascontiguousarray(np.asarray(inputs[k], dtype=np.float32))
        for k in ("u_in_w", "u_in_b", "u_out_w", "u_out_b",
                  "i_in_w", "i_in_b", "i_out_w", "i_out_b",
                  "W1", "b1", "W2", "b2", "W3", "b3")
    }

    nc = _get_program()
    in_maps = []
    for c in range(N_CORES):
        m = {"cat_table": cat, "idx": idx_all[c * BC:(c + 1) * BC]}
        m.update(weights)
        in_maps.append(m)

    res = run_bass_kernel_spmd(nc, in_maps, core_ids=list(range(N_CORES)))
    out = np.concatenate([res.results[c]["y"] for c in range(N_CORES)])
    return out.astype(np.float32)


if __name__ == "__main__":
    # smoke test with random data (no reference available here)
    rng = np.random.default_rng(0)
    demo = {
        "user": rng.integers(0, V, size=(BATCH,)),
        "item": rng.integers(0, V, size=(BATCH,)),
        "user_table": rng.standard_normal((V, EMB)).astype(np.float32) * 0.1,
        "item_table": rng.standard_normal((V, EMB)).astype(np.float32) * 0.1,
        "user_topk": rng.integers(0, V, size=(V, K)),
        "item_topk": rng.integers(0, V, size=(V, K)),
    }
    s = 1.0 / np.sqrt(EMB)
    for sd in ("u", "i"):
        demo[f"{sd}_in_w"] = rng.uniform(-s, s, (3 * EMB, EMB)).astype(np.float32)
        demo[f"{sd}_in_b"] = np.zeros(3 * EMB, np.float32)
        demo[f"{sd}_out_w"] = rng.uniform(-s, s, (EMB, EMB)).astype(np.float32)
        demo[f"{sd}_out_b"] = np.zeros(EMB, np.float32)
    demo["W1"] = rng.uniform(-0.06, 0.06, (128, 256)).astype(np.float32)
    demo["b1"] = np.zeros(128, np.float32)
    demo["W2"] = rng.uniform(-0.09, 0.09, (64, 128)).astype(np.float32)
    demo["b2"] = np.zeros(64, np.float32)
    demo["W3"] = rng.uniform(-0.125, 0.125, (1, 64)).astype(np.float32)
    demo["b3"] = np.zeros(1, np.float32)
    y = kernel(**demo)
    print("kernel output:", y.shape, y.dtype, y[:4])


# revision 4
# speedup vs baseline: 1.0926x; 1.0926x over previous
"""NeighborAware GNN message-passing kernel for 8 Trainium2 NeuronCores.

Strategy (data-parallel): shard the 16384-sample batch across 8 cores
(2048 samples each); replicate the embedding tables + tiny weights.

Algebraic collapse of the single-head attention (softmax is shift
invariant, and only the first-token output is used):
    scores_j = x0^T A x_j + c1 . x_j        A  = Wq^T Wk / sqrt(E)
                                            c1 = Wk^T bq / sqrt(E)
    ctx_out  = (sum_j a_j x_j) @ M_vo + b'  M_vo = (Wo Wv)^T
so neighbor embeddings never need projection; each sample needs only its
12 gathered rows (2 sides x (target + 5 neighbors)), per-sample dot
products on DVE, and two small matmuls per 128-sample tile. The output
biases (Wo bv + out_b) are folded into the first MLP bias.

Per 128-sample tile-and-side: 6 indirect DMA gathers ([P,1]-offset form,
the only HW-supported one), one PE transpose of the target rows, the
z0 = x0 A + c1 matmul, 6 fused mul-reduce score ops (custom-DVE
TENSOR_TENSOR_REDUCE with the pad mask as the accumulator seed),
softmax via ACT Exp with fused bias/accum, 6 scaled copies + 6
accumulating PE transposes for the weighted sum, and one matmul for the
projected context. A second phase runs the 3-layer MLP transposed so no
further transposes are needed.
"""
import sys

if "/opt/trn_rl_repo" not in sys.path:
    sys.path.insert(0, "/opt/trn_rl_repo")

import numpy as np

import concourse.bass as bass
import concourse.bacc as bacc
import concourse.tile as tile
from concourse import mybir
from concourse.masks import make_identity
from concourse.dve_ops import TENSOR_TENSOR_REDUCE
from concourse.bass_utils import run_bass_kernel_spmd

N_CORES = 8
BATCH = 16384
BC = BATCH // N_CORES          # 2048 samples per core
P = 128
NTILES = BC // P               # 16 tiles per core
EMB = 128
K = 5
NJ = K + 1                     # target + 5 neighbors
V = 100001                     # rows per table (incl. padding row 0)
CATV = 2 * V                   # user and item tables concatenated

f32 = mybir.dt.float32
i32 = mybir.dt.int32
RSQRT_E = float(1.0 / np.sqrt(np.float32(EMB)))

_PROGRAM = None


def _build_program():
    nc = bacc.Bacc()

    cat_d = nc.dram_tensor("cat_table", [CATV, EMB], f32, kind="ExternalInput")
    idx_d = nc.dram_tensor("idx", [BC, 2 * NJ], i32, kind="ExternalInput")
    wdram = {}
    for s in ("u", "i"):
        wdram[f"{s}_in_w"] = nc.dram_tensor(f"{s}_in_w", [3 * EMB, EMB], f32, kind="ExternalInput")
        wdram[f"{s}_in_b"] = nc.dram_tensor(f"{s}_in_b", [3 * EMB], f32, kind="ExternalInput")
        wdram[f"{s}_out_w"] = nc.dram_tensor(f"{s}_out_w", [EMB, EMB], f32, kind="ExternalInput")
        wdram[f"{s}_out_b"] = nc.dram_tensor(f"{s}_out_b", [EMB], f32, kind="ExternalInput")
    W1_d = nc.dram_tensor("W1", [EMB, 2 * EMB], f32, kind="ExternalInput")
    b1_d = nc.dram_tensor("b1", [EMB], f32, kind="ExternalInput")
    W2_d = nc.dram_tensor("W2", [EMB // 2, EMB], f32, kind="ExternalInput")
    b2_d = nc.dram_tensor("b2", [EMB // 2], f32, kind="ExternalInput")
    W3_d = nc.dram_tensor("W3", [1, EMB // 2], f32, kind="ExternalInput")
    b3_d = nc.dram_tensor("b3", [1], f32, kind="ExternalInput")
    y_d = nc.dram_tensor("y", [BC], f32, kind="ExternalOutput")

    with tile.TileContext(nc) as tc:
        with tc.tile_pool(name="singles", bufs=1) as singles:
            ident = singles.tile([P, P], f32)
            make_identity(nc, ident[:])
            ones_row = singles.tile([1, P], f32)
            nc.vector.memset(ones_row[:], 1.0)

            # ---- load raw weights -------------------------------------
            A_s, c1_s, Mvo_s, bout_s = [], [], [], []
            with tc.tile_pool(name="wload", bufs=1) as wl, \
                 tc.tile_pool(name="setup_psum", bufs=1, space="PSUM") as sps:
                for si, s in enumerate(("u", "i")):
                    wq = wl.tile([P, P], f32, tag=f"wq{s}")
                    wk = wl.tile([P, P], f32, tag=f"wk{s}")
                    wv = wl.tile([P, P], f32, tag=f"wv{s}")
                    nc.sync.dma_start(out=wq[:], in_=wdram[f"{s}_in_w"][0:P, :])
                    nc.sync.dma_start(out=wk[:], in_=wdram[f"{s}_in_w"][P:2 * P, :])
                    nc.sync.dma_start(out=wv[:], in_=wdram[f"{s}_in_w"][2 * P:3 * P, :])
                    bq = wl.tile([P, 1], f32, tag=f"bq{s}")
                    bv = wl.tile([P, 1], f32, tag=f"bv{s}")
                    nc.sync.dma_start(out=bq[:], in_=wdram[f"{s}_in_b"][0:P, None])
                    nc.sync.dma_start(out=bv[:], in_=wdram[f"{s}_in_b"][2 * P:3 * P, None])
                    wo = wl.tile([P, P], f32, tag=f"wo{s}")
                    nc.sync.dma_start(out=wo[:], in_=wdram[f"{s}_out_w"][:, :])
                    outb = wl.tile([P, 1], f32, tag=f"ob{s}")
                    nc.sync.dma_start(out=outb[:], in_=wdram[f"{s}_out_b"][:, None])

                    # A = Wq^T Wk / sqrt(E)   [e, e']
                    A_p = sps.tile([P, P], f32, tag="sp0")
                    nc.tensor.matmul(A_p[:], lhsT=wq[:], rhs=wk[:], start=True, stop=True)
                    A_t = singles.tile([P, P], f32, tag=f"A{s}")
                    nc.vector.tensor_scalar_mul(A_t[:], A_p[:], RSQRT_E)
                    A_s.append(A_t)

                    # c1 = bq^T Wk / sqrt(E)  [1, e']
                    c1_p = sps.tile([1, P], f32, tag="sp1")
                    nc.tensor.matmul(c1_p[:], lhsT=bq[:], rhs=wk[:], start=True, stop=True)
                    c1_t = singles.tile([1, P], f32, tag=f"c1{s}")
                    nc.vector.tensor_scalar_mul(c1_t[:], c1_p[:], RSQRT_E)
                    c1_s.append(c1_t)

                    # WoT [g, f]
                    woT_p = sps.tile([P, P], f32, tag="sp0")
                    nc.tensor.transpose(woT_p[:], wo[:], ident[:])
                    woT = wl.tile([P, P], f32, tag=f"woT{s}")
                    nc.vector.tensor_copy(woT[:], woT_p[:])

                    # M_vo[e, f] = sum_g Wv[g,e] WoT[g,f]
                    mvo_p = sps.tile([P, P], f32, tag="sp0")
                    nc.tensor.matmul(mvo_p[:], lhsT=wv[:], rhs=woT[:], start=True, stop=True)
                    mvo = singles.tile([P, P], f32, tag=f"mvo{s}")
                    nc.vector.tensor_copy(mvo[:], mvo_p[:])
                    Mvo_s.append(mvo)

                    # b_out = Wo bv + out_b  [f, 1]
                    bo_p = sps.tile([P, 1], f32, tag="sp1")
                    nc.tensor.matmul(bo_p[:], lhsT=woT[:], rhs=bv[:], start=True, stop=True)
                    bo = wl.tile([P, 1], f32, tag=f"bo{s}")
                    nc.vector.tensor_add(out=bo[:], in0=bo_p[:], in1=outb[:])
                    bout_s.append(bo)

                # MLP weights (transposed for lhsT use)
                w1 = wl.tile([P, 2 * P], f32)
                nc.sync.dma_start(out=w1[:], in_=W1_d[:, :])
                w1uT_p = sps.tile([P, P], f32, tag="sp0")
                nc.tensor.transpose(w1uT_p[:], w1[:, 0:P], ident[:])
                w1uT = singles.tile([P, P], f32)
                nc.vector.tensor_copy(w1uT[:], w1uT_p[:])
                w1iT_p = sps.tile([P, P], f32, tag="sp0")
                nc.tensor.transpose(w1iT_p[:], w1[:, P:2 * P], ident[:])
                w1iT = singles.tile([P, P], f32)
                nc.vector.tensor_copy(w1iT[:], w1iT_p[:])

                w2 = wl.tile([P // 2, P], f32)
                nc.sync.dma_start(out=w2[:], in_=W2_d[:, :])
                w2T_p = sps.tile([P, P // 2], f32, tag="sp0")
                nc.tensor.matmul(w2T_p[:], lhsT=w2[:], rhs=ident[0:P // 2, 0:P // 2],
                                 is_transpose=True, start=True, stop=True)
                w2T = singles.tile([P, P // 2], f32)
                nc.vector.tensor_copy(w2T[:], w2T_p[:])

                w3c = singles.tile([P // 2, 1], f32)
                nc.sync.dma_start(out=w3c[:], in_=W3_d[0, :, None])
                b1c = wl.tile([P, 1], f32)
                nc.sync.dma_start(out=b1c[:], in_=b1_d[:, None])
                b2c = singles.tile([P // 2, 1], f32)
                nc.sync.dma_start(out=b2c[:], in_=b2_d[:, None])
                b3c = singles.tile([1, 1], f32)
                nc.sync.dma_start(out=b3c[:], in_=b3_d[:, None])

                # b1' = b1 + W1u b_out_u + W1i b_out_i
                b1p_p = sps.tile([P, 1], f32, tag="sp1")
                nc.tensor.matmul(b1p_p[:], lhsT=w1uT[:], rhs=bout_s[0][:], start=True, stop=False)
                nc.tensor.matmul(b1p_p[:], lhsT=w1iT[:], rhs=bout_s[1][:], start=False, stop=True)
                b1p = singles.tile([P, 1], f32)
                nc.vector.tensor_add(out=b1p[:], in0=b1p_p[:], in1=b1c[:])

            # context staging for phase B: [side*16 + t] slots of [P, P]
            ctx_all = singles.tile([P, 2 * NTILES, P], f32)
            y_row = singles.tile([1, BC], f32)

            # ------------- main loop: gather + attention + MLP -------------
            with tc.tile_pool(name="idxp", bufs=4) as idxp, \
                 tc.tile_pool(name="gp", bufs=6) as gp, \
                 tc.tile_pool(name="wp", bufs=3) as wp, \
                 tc.tile_pool(name="sp", bufs=4) as sp, \
                 tc.tile_pool(name="cp", bufs=4) as cp, \
                 tc.tile_pool(name="pa", bufs=2, space="PSUM") as pa:
                for t in range(NTILES):
                    idx_t = idxp.tile([P, 2 * NJ], i32)
                    nc.sync.dma_start(out=idx_t[:], in_=idx_d[t * P:(t + 1) * P, :])
                    for side in range(2):
                        base = side * NJ
                        xg = gp.tile([P, NJ, EMB], f32, tag=f"xg{side}")
                        for j in range(NJ):
                            nc.gpsimd.indirect_dma_start(
                                out=xg[:, j, :], out_offset=None, in_=cat_d[:, :],
                                in_offset=bass.IndirectOffsetOnAxis(
                                    ap=idx_t[:, base + j:base + j + 1], axis=0))

                        x0T_p = pa.tile([P, P], f32, tag="x0T")
                        nc.tensor.transpose(x0T_p[:], xg[:, 0, :], ident[:])
                        x0T = cp.tile([P, P], f32, tag="x0T_s")
                        nc.vector.tensor_copy(x0T[:], x0T_p[:])

                        z0_p = pa.tile([P, P], f32, tag="z0")
                        nc.tensor.matmul(z0_p[:], lhsT=x0T[:], rhs=A_s[side][:],
                                         start=True, stop=False)
                        nc.tensor.matmul(z0_p[:], lhsT=ones_row[:], rhs=c1_s[side][:],
                                         start=False, stop=True)

                        msk = sp.tile([P, K], f32, tag="msk")
                        nc.vector.tensor_scalar(
                            out=msk[:], in0=idx_t[:, base + 1:base + NJ],
                            scalar1=0, scalar2=-1e30,
                            op0=mybir.AluOpType.is_equal, op1=mybir.AluOpType.mult)

                        scores = sp.tile([P, NJ], f32, tag="sc")
                        scratch = cp.tile([P, P], f32, tag="ttr")
                        for j in range(NJ):
                            nc.vector._custom_dve(
                                TENSOR_TENSOR_REDUCE,
                                out=scratch[:], in0=z0_p[:], in1=xg[:, j, :],
                                s0=(0.0 if j == 0 else msk[:, j - 1:j]), s1=1.0,
                                accum_out=scores[:, j:j + 1])

                        negmx = sp.tile([P, 1], f32, tag="mx")
                        nc.vector.reduce_max(out=negmx[:], in_=scores[:],
                                             axis=mybir.AxisListType.X, negate=True)
                        aexp = sp.tile([P, NJ], f32, tag="ae")
                        sumex = sp.tile([P, 1], f32, tag="se")
                        nc.scalar.activation(out=aexp[:], in_=scores[:],
                                             func=mybir.ActivationFunctionType.Exp,
                                             bias=negmx[:], scale=1.0, accum_out=sumex[:])
                        rec = sp.tile([P, 1], f32, tag="rc")
                        nc.vector.reciprocal(rec[:], sumex[:])
                        anorm = sp.tile([P, NJ], f32, tag="an")
                        nc.vector.tensor_scalar_mul(anorm[:], aexp[:], rec[:])

                        wacc = wp.tile([P, NJ, EMB], f32, tag=f"wacc{side}")
                        for j in range(NJ):
                            nc.vector.tensor_scalar_mul(wacc[:, j, :], xg[:, j, :],
                                                        anorm[:, j:j + 1])
                        wT_p = pa.tile([P, P], f32, tag="wT")
                        for j in range(NJ):
                            nc.tensor.matmul(wT_p[:], lhsT=wacc[:, j, :], rhs=ident[:],
                                             is_transpose=True,
                                             start=(j == 0), stop=(j == NJ - 1))
                        wT = cp.tile([P, P], f32, tag="wT_s")
                        nc.vector.tensor_copy(wT[:], wT_p[:])

                        ctx_p = pa.tile([P, P], f32, tag="ctx")
                        nc.tensor.matmul(ctx_p[:], lhsT=Mvo_s[side][:], rhs=wT[:],
                                         start=True, stop=True)
                        nc.vector.tensor_copy(ctx_all[:, side * NTILES + t, :], ctx_p[:])

                    # MLP for this tile, inline (reuses phase-A PSUM tags so
                    # the scheduler can interleave it under the gather stream)
                    h1_p = pa.tile([P, P], f32, tag="x0T")
                    nc.tensor.matmul(h1_p[:], lhsT=w1uT[:], rhs=ctx_all[:, t, :],
                                     start=True, stop=False)
                    nc.tensor.matmul(h1_p[:], lhsT=w1iT[:], rhs=ctx_all[:, NTILES + t, :],
                                     start=False, stop=True)
                    h1 = cp.tile([P, P], f32, tag="h1s")
                    nc.scalar.activation(out=h1[:], in_=h1_p[:],
                                         func=mybir.ActivationFunctionType.Relu,
                                         bias=b1p[:], scale=1.0)
                    h2_p = pa.tile([P // 2, P], f32, tag="z0")
                    nc.tensor.matmul(h2_p[:], lhsT=w2T[:], rhs=h1[:], start=True, stop=True)
                    h2 = cp.tile([P // 2, P], f32, tag="h2s")
                    nc.scalar.activation(out=h2[:], in_=h2_p[:],
                                         func=mybir.ActivationFunctionType.Relu,
                                         bias=b2c[:], scale=1.0)
                    y_p = pa.tile([1, P], f32, tag="wT")
                    nc.tensor.matmul(y_p[:], lhsT=w3c[:], rhs=h2[:], start=True, stop=True)
                    nc.vector.tensor_scalar_add(y_row[:, t * P:(t + 1) * P], y_p[:], b3c[:])

            nc.sync.dma_start(out=y_d[None, :], in_=y_row[:])

    nc.compile()
    return nc


def _get_program():
    global _PROGRAM
    if _PROGRAM is None:
        _PROGRAM = _build_program()
    return _PROGRAM


def kernel(**inputs) -> np.ndarray:
    user = np.asarray(inputs["user"]).astype(np.int64)
    item = np.asarray(inputs["item"]).astype(np.int64)
    user_table = np.ascontiguousarray(np.asarray(inputs["user_table"], dtype=np.float32))
    item_table = np.ascontiguousarray(np.asarray(inputs["item_table"], dtype=np.float32))
    user_topk = np.asarray(inputs["user_topk"]).astype(np.int64)
    item_topk = np.asarray(inputs["item_topk"]).astype(np.int64)

    nv = user_table.shape[0]
    assert nv == V and user.shape[0] == BATCH, (user_table.shape, user.shape)

    cat = np.ascontiguousarray(np.concatenate([user_table, item_table], axis=0))

    # index preprocessing: resolve top-k neighbor ids for the batch and
    # fold the item-table offset in; id 0 stays 0 (padding row, masked out).
    u_ids = user_topk[user]                                   # [B, K]
    i_ids_raw = item_topk[item]                               # [B, K]
    i_ids = np.where(i_ids_raw == 0, 0, i_ids_raw + nv)
    idx_all = np.concatenate(
        [user[:, None], u_ids, item[:, None] + nv, i_ids], axis=1
    ).astype(np.int32)                                        # [B, 12]

    weights = {
        k: np.ascontiguousarray(np.asarray(inputs[k], dtype=np.float32))
        for k in ("u_in_w", "u_in_b", "u_out_w", "u_out_b",
                  "i_in_w", "i_in_b", "i_out_w", "i_out_b",
                  "W1", "b1", "W2", "b2", "W3", "b3")
    }

    nc = _get_program()
    in_maps = []
    for c in range(N_CORES):
        m = {"cat_table": cat, "idx": idx_all[c * BC:(c + 1) * BC]}
        m.update(weights)
        in_maps.append(m)

    res = run_bass_kernel_spmd(nc, in_maps, core_ids=list(range(N_CORES)))
    out = np.concatenate([res.results[c]["y"] for c in range(N_CORES)])
    return out.astype(np.float32)


if __name__ == "__main__":
    # smoke test with random data (no reference available here)
    rng = np.random.default_rng(0)
    demo = {
        "user": rng.integers(0, V, size=(BATCH,)),
        "item": rng.integers(0, V, size=(BATCH,)),
        "user_table": rng.standard_normal((V, EMB)).astype(np.float32) * 0.1,
        "item_table": rng.standard_normal((V, EMB)).astype(np.float32) * 0.1,
        "user_topk": rng.integers(0, V, size=(V, K)),
        "item_topk": rng.integers(0, V, size=(V, K)),
    }
    s = 1.0 / np.sqrt(EMB)
    for sd in ("u", "i"):
        demo[f"{sd}_in_w"] = rng.uniform(-s, s, (3 * EMB, EMB)).astype(np.float32)
        demo[f"{sd}_in_b"] = np.zeros(3 * EMB, np.float32)
        demo[f"{sd}_out_w"] = rng.uniform(-s, s, (EMB, EMB)).astype(np.float32)
        demo[f"{sd}_out_b"] = np.zeros(EMB, np.float32)
    demo["W1"] = rng.uniform(-0.06, 0.06, (128, 256)).astype(np.float32)
    demo["b1"] = np.zeros(128, np.float32)
    demo["W2"] = rng.uniform(-0.09, 0.09, (64, 128)).astype(np.float32)
    demo["b2"] = np.zeros(64, np.float32)
    demo["W3"] = rng.uniform(-0.125, 0.125, (1, 64)).astype(np.float32)
    demo["b3"] = np.zeros(1, np.float32)
    y = kernel(**demo)
    print("kernel output:", y.shape, y.dtype, y[:4])
